# revision 27
# baseline (speedup 1.0000x reference)
"""BinsChamferLoss Trainium2 kernel.

Computes mean over batch of (cham_x + cham_y) where, per batch row:
  cham_y = sum over valid pixels y of min_b (bin_b - y)^2 / max(count_valid, 1)
  cham_x = mean over 256 bins of min over valid pixels y of (bin_b - y)^2
A pixel is valid iff depth >= 1e-3. The sort in the reference is irrelevant:
both terms are set-based reductions over the same (bin, pixel) distance matrix.

Strategy: 8-way data parallel over pixels (all 4 batch rows on every core,
1/8 of the pixels each). Per 128-pixel group: ScalarE computes the full
[128 pixels x 256 bins] d2 tile as Square(bins + (-y_p)) (exact
subtract-then-square via the activation bias path), VectorE reduce_min over
bins gives the per-pixel nearest-bin distance, and a running elementwise min
accumulates the per-bin nearest-pixel distance. Invalid pixels are shifted by
+1e6 so they can never win a min; they are masked out of the cham_y sum.
Host-side: tiny fp64 combine of per-core partials.
"""
import os
import sys
import types

sys.path.insert(0, "/opt/trn_rl_repo")

import numpy as np

N_ROWS = 4
N_BINS = 256
HW = 240 * 320            # 76800 pixels per row
N_CORES = 8
PX_PER_CORE = HW // N_CORES   # 9600
FREE = PX_PER_CORE // 128     # 75 pixel-groups per core per row
MIN_DEPTH = 1e-3
BIG = 1e10
OFF = 1e6


def _install_ntff_hook_shim():
    """Register the axon NTFF profiling hook if the antenv module lacks it."""
    try:
        from antenv import axon_hooks  # noqa: F401
        return
    except ImportError:
        pass
    try:
        from trn_agent_boot.trn_boot import _ntff_profile_via_ctypes
        hook = _ntff_profile_via_ctypes("/opt/axon/libaxon_pjrt.so")
    except Exception:
        hook = None
    mod = types.ModuleType("antenv.axon_hooks")
    mod._hook = hook
    mod.get_axon_ntff_profile_hook = lambda: mod._hook

    def set_axon_ntff_profile_hook(h):
        mod._hook = h

    mod.set_axon_ntff_profile_hook = set_axon_ntff_profile_hook
    sys.modules["antenv.axon_hooks"] = mod
    import antenv
    antenv.axon_hooks = mod


def _patch_tile_drain_split():
    """Walrus's CoreV3 codegen rejects >1 sync wait on a Drain; Tile's tail
    drain waits on every live semaphore. Split the waits across a chain of
    drain instructions (1 wait each)."""
    import bass_rust
    import concourse.tile as tile
    from concourse.vector_clock import ScopedClock

    if getattr(tile.TileContext._drain_and_barrier, "_split_patched", False):
        return

    def _drain_and_barrier(self, tick_clock, wait_clock):
        nc = self.nc
        drain_inst = nc.sync.drain()
        wait_clock.add_sem_waits(
            drain_inst.ins, ScopedClock({None: tick_clock.global_clock})
        )
        si = drain_inst.ins.sync_info
        if si is not None and len(si.on_wait) > 1:
            waits = list(si.on_wait)
            drain_inst.ins.sync_info = bass_rust.SyncInfo(
                on_wait=waits[:1], on_update=list(si.on_update)
            )
            for i in range(1, len(waits)):
                extra = nc.sync.drain()
                extra.ins.sync_info = bass_rust.SyncInfo(
                    on_wait=waits[i : i + 1], on_update=[]
                )
        nc.all_engine_barrier()
        popped = nc._tile_sem_poison_stack.pop()
        assert popped is self._sem_poison
        nc.clear_and_free_semaphores(list(self.sems.allocated().values()))
        nc.all_engine_barrier()

    _drain_and_barrier._split_patched = True
    tile.TileContext._drain_and_barrier = _drain_and_barrier


def _split_excess_waits(nc, max_waits=1):
    """Walrus's codegen rejects instructions carrying more than one sync wait.
    Move excess waits onto pure-wait EventSemaphore instructions inserted
    immediately before the over-subscribed instruction on the same engine."""
    import bass_rust
    from concourse import mybir

    n_split = 0
    for f in nc.m.functions:
        for bb in f.blocks:
            lst = bb.instructions
            i = 0
            while i < len(lst):
                ins = lst[i]
                si = getattr(ins, "sync_info", None)
                if si is not None and len(si.on_wait) > max_waits:
                    waits = list(si.on_wait)
                    ins.sync_info = bass_rust.SyncInfo(
                        on_wait=waits[:max_waits], on_update=list(si.on_update)
                    )
                    for j, w in enumerate(waits[max_waits:]):
                        ev = mybir.InstEventSemaphore(
                            name=f"{ins.name}-xw{j}", ins=[], outs=[]
                        )
                        ev.engine = ins.engine
                        ev.sync_info = bass_rust.SyncInfo(on_wait=[w], on_update=[])
                        lst.insert(i, ev)
                        i += 1
                    n_split += 1
                i += 1
    return n_split


_NC_CACHE = None


def _build_module():
    global _NC_CACHE
    if _NC_CACHE is not None:
        return _NC_CACHE

    _install_ntff_hook_shim()
    _patch_tile_drain_split()

    import concourse.bass as bass
    import concourse.tile as tile
    from concourse import mybir

    f32 = mybir.dt.float32
    bf16 = mybir.dt.bfloat16
    Alu = mybir.AluOpType
    Act = mybir.ActivationFunctionType

    nc = bass.Bass("TRN2", target_bir_lowering=False, debug=False)
    bins_d = nc.dram_tensor("bins", [N_ROWS, N_BINS], f32, kind="ExternalInput").ap()
    px_d = nc.dram_tensor("px", [N_ROWS, 128, FREE], f32, kind="ExternalInput").ap()
    runmin_d = nc.dram_tensor(
        "runmin", [N_ROWS, 128, N_BINS], f32, kind="ExternalOutput"
    ).ap()
    nnsum_d = nc.dram_tensor("nnsum", [N_ROWS, 128, 1], f32, kind="ExternalOutput").ap()
    cnt_d = nc.dram_tensor("cnt", [N_ROWS, 128, 1], f32, kind="ExternalOutput").ap()

    # ScalarE produces |bins - y| tiles (Abs activation with per-partition
    # bias); VectorE does both reductions, batched K slots per instruction so
    # the 58-cycle DVE instruction overhead amortizes.
    KB = 25  # slots per batched DVE op; FREE must be divisible by KB

    with tile.TileContext(nc) as tc:
        with (
            tc.tile_pool(name="row", bufs=3) as row_pool,
            tc.tile_pool(name="acc", bufs=3) as acc_pool,
            tc.tile_pool(name="d2p", bufs=6) as d2_pool,
            tc.tile_pool(name="small", bufs=4) as small_pool,
        ):
            for r in range(N_ROWS):
                bins_bc = row_pool.tile([128, N_BINS], f32, tag="bins_bc")
                bins_row = bins_d[r]
                bins_bcast_ap = bass.AP(
                    tensor=bins_row.tensor,
                    offset=bins_row.offset,
                    ap=[[0, 128]] + list(bins_row.ap),
                )
                nc.sync.dma_start(out=bins_bc[:], in_=bins_bcast_ap)

                y = row_pool.tile([128, FREE], f32, tag="y")
                nc.sync.dma_start(out=y[:], in_=px_d[r])

                # mask = (y >= MIN_DEPTH) in {0.0, 1.0}
                mask = row_pool.tile([128, FREE], f32, tag="mask")
                nc.vector.tensor_scalar(
                    out=mask[:], in0=y[:], scalar1=MIN_DEPTH, scalar2=None,
                    op0=Alu.is_ge,
                )
                # y_off = y + OFF*(1-mask); nyo = -y_off
                t1 = small_pool.tile([128, FREE], f32, tag="t1")
                nc.vector.tensor_scalar(
                    out=t1[:], in0=mask[:], scalar1=OFF, scalar2=OFF,
                    op0=Alu.mult, op1=Alu.subtract,
                )
                nyo = row_pool.tile([128, FREE], f32, tag="nyo")
                nc.vector.tensor_tensor(out=nyo[:], in0=t1[:], in1=y[:], op=Alu.subtract)


                rm = acc_pool.tile([128, N_BINS], f32, tag="rm")
                nc.vector.memset(rm[:], BIG)
                nn_all = acc_pool.tile([128, FREE], f32, tag="nn_all")

                # Small leading batches on the first row shorten the pipeline
                # fill before VectorE gets its first work.
                if r == 0:
                    schedule = [5, 10, 15, 20, 25]
                elif r == N_ROWS - 1:
                    schedule = [25, 20, 15, 10, 5]
                else:
                    schedule = [25, 25, 25]
                f0 = 0
                for bsz in schedule:
                    adw = d2_pool.tile([128, KB, N_BINS], f32, tag="adw")
                    for k in range(bsz):
                        f = f0 + k
                        nc.scalar.activation(
                            out=adw[:, k, :], in_=bins_bc[:], func=Act.Abs,
                            bias=nyo[:, f : f + 1], scale=1.0,
                        )
                    # per-pixel nearest-bin |d|: reduce innermost (bins) for
                    # the batch's slots in one DVE op
                    nc.vector.tensor_reduce(
                        out=nn_all[:, f0 : f0 + bsz], in_=adw[:, 0:bsz, :],
                        axis=mybir.AxisListType.X, op=Alu.min,
                    )
                    # per-bin running min: in-place pairwise halving tree over
                    # the batch's slots, then one fold into rm
                    n = bsz
                    while n > 1:
                        h = n // 2
                        nc.vector.tensor_tensor(
                            out=adw[:, 0:h, :], in0=adw[:, 0:h, :],
                            in1=adw[:, h : 2 * h, :], op=Alu.min,
                        )
                        if n % 2:
                            nc.vector.tensor_tensor(
                                out=adw[:, 0, :], in0=adw[:, 0, :],
                                in1=adw[:, n - 1, :], op=Alu.min,
                            )
                        n = h
                    nc.vector.tensor_tensor(
                        out=rm[:], in0=rm[:], in1=adw[:, 0, :], op=Alu.min
                    )
                    f0 += bsz

                # cham_y partials: nn^2 * mask summed over the free dim + count
                nn2 = small_pool.tile([128, FREE], f32, tag="nn2")
                nc.scalar.activation(out=nn2[:], in_=nn_all[:], func=Act.Square)
                nnm = small_pool.tile([128, FREE], f32, tag="nnm")
                nc.vector.tensor_tensor(out=nnm[:], in0=nn2[:], in1=mask[:], op=Alu.mult)
                nnsum = small_pool.tile([128, 1], f32, tag="nnsum")
                nc.vector.tensor_reduce(
                    out=nnsum[:], in_=nnm[:], axis=mybir.AxisListType.X, op=Alu.add
                )
                cnt = small_pool.tile([128, 1], f32, tag="cnt")
                nc.vector.tensor_reduce(
                    out=cnt[:], in_=mask[:], axis=mybir.AxisListType.X, op=Alu.add
                )

                nc.sync.dma_start(out=runmin_d[r], in_=rm[:])
                nc.sync.dma_start(out=nnsum_d[r], in_=nnsum[:])
                nc.sync.dma_start(out=cnt_d[r], in_=cnt[:])

    _split_excess_waits(nc)
    _NC_CACHE = nc
    return nc


LAST_RESULTS = None


def kernel(bin_centers: np.ndarray, target_depth_maps: np.ndarray) -> np.ndarray:
    global LAST_RESULTS
    nc = _build_module()
    from concourse import bass_utils

    trace = bool(os.environ.get("KERNEL_TRACE"))
    if trace:
        bass_utils.upload_artifacts = lambda tmpdir: "local://" + str(tmpdir)

    bins = np.ascontiguousarray(bin_centers, dtype=np.float32)
    tp = np.ascontiguousarray(
        np.asarray(target_depth_maps, dtype=np.float32).reshape(N_ROWS, HW)
    )

    in_maps = []
    for c in range(N_CORES):
        sl = tp[:, c * PX_PER_CORE : (c + 1) * PX_PER_CORE].reshape(N_ROWS, 128, FREE)
        in_maps.append({"bins": bins, "px": np.ascontiguousarray(sl)})

    res = bass_utils.run_bass_kernel_spmd(
        nc, in_maps, core_ids=list(range(N_CORES)), trace=trace
    )
    LAST_RESULTS = res

    runmin = np.stack([r["runmin"] for r in res.results])  # [8, 4, 128, 256]
    nnsum = np.stack([r["nnsum"] for r in res.results])    # [8, 4, 128, 1]
    cnt = np.stack([r["cnt"] for r in res.results])        # [8, 4, 128, 1]

    # runmin holds |d|; square in fp32 (monotone => same as min over fp32 d^2),
    # then clamp to BIG to reproduce the reference's invalid-pixel sentinel.
    per_bin_absd = runmin.min(axis=(0, 2)).astype(np.float32)       # [4, 256]
    per_bin = np.minimum(per_bin_absd * per_bin_absd, np.float32(BIG))
    cham_x = per_bin.mean(axis=1, dtype=np.float64)                 # [4]
    lengths = cnt.sum(axis=(0, 2, 3), dtype=np.float64)             # [4]
    sums = nnsum.sum(axis=(0, 2, 3), dtype=np.float64)              # [4]
    cham_y = sums / np.maximum(lengths, 1.0)
    out = np.mean(cham_x + cham_y)
    return np.asarray(out, dtype=np.float32)


# revision 28
# speedup vs baseline: 1.0198x; 1.0198x over previous
"""BinsChamferLoss Trainium2 kernel.

Computes mean over batch of (cham_x + cham_y) where, per batch row:
  cham_y = sum over valid pixels y of min_b (bin_b - y)^2 / max(count_valid, 1)
  cham_x = mean over 256 bins of min over valid pixels y of (bin_b - y)^2
A pixel is valid iff depth >= 1e-3. The sort in the reference is irrelevant:
both terms are set-based reductions over the same (bin, pixel) distance matrix.

Strategy: 8-way data parallel over pixels (all 4 batch rows on every core,
1/8 of the pixels each). Per 128-pixel group: ScalarE computes the full
[128 pixels x 256 bins] d2 tile as Square(bins + (-y_p)) (exact
subtract-then-square via the activation bias path), VectorE reduce_min over
bins gives the per-pixel nearest-bin distance, and a running elementwise min
accumulates the per-bin nearest-pixel distance. Invalid pixels are shifted by
+1e6 so they can never win a min; they are masked out of the cham_y sum.
Host-side: tiny fp64 combine of per-core partials.
"""
import os
import sys
import types

sys.path.insert(0, "/opt/trn_rl_repo")

import numpy as np

N_ROWS = 4
N_BINS = 256
HW = 240 * 320            # 76800 pixels per row
N_CORES = 8
PX_PER_CORE = HW // N_CORES   # 9600
FREE = PX_PER_CORE // 128     # 75 pixel-groups per core per row
MIN_DEPTH = 1e-3
BIG = 1e10
OFF = 1e6


def _install_ntff_hook_shim():
    """Register the axon NTFF profiling hook if the antenv module lacks it."""
    try:
        from antenv import axon_hooks  # noqa: F401
        return
    except ImportError:
        pass
    try:
        from trn_agent_boot.trn_boot import _ntff_profile_via_ctypes
        hook = _ntff_profile_via_ctypes("/opt/axon/libaxon_pjrt.so")
    except Exception:
        hook = None
    mod = types.ModuleType("antenv.axon_hooks")
    mod._hook = hook
    mod.get_axon_ntff_profile_hook = lambda: mod._hook

    def set_axon_ntff_profile_hook(h):
        mod._hook = h

    mod.set_axon_ntff_profile_hook = set_axon_ntff_profile_hook
    sys.modules["antenv.axon_hooks"] = mod
    import antenv
    antenv.axon_hooks = mod


def _patch_tile_drain_split():
    """Walrus's CoreV3 codegen rejects >1 sync wait on a Drain; Tile's tail
    drain waits on every live semaphore. Split the waits across a chain of
    drain instructions (1 wait each)."""
    import bass_rust
    import concourse.tile as tile
    from concourse.vector_clock import ScopedClock

    if getattr(tile.TileContext._drain_and_barrier, "_split_patched", False):
        return

    def _drain_and_barrier(self, tick_clock, wait_clock):
        nc = self.nc
        drain_inst = nc.sync.drain()
        wait_clock.add_sem_waits(
            drain_inst.ins, ScopedClock({None: tick_clock.global_clock})
        )
        si = drain_inst.ins.sync_info
        if si is not None and len(si.on_wait) > 1:
            waits = list(si.on_wait)
            drain_inst.ins.sync_info = bass_rust.SyncInfo(
                on_wait=waits[:1], on_update=list(si.on_update)
            )
            for i in range(1, len(waits)):
                extra = nc.sync.drain()
                extra.ins.sync_info = bass_rust.SyncInfo(
                    on_wait=waits[i : i + 1], on_update=[]
                )
        nc.all_engine_barrier()
        popped = nc._tile_sem_poison_stack.pop()
        assert popped is self._sem_poison
        nc.clear_and_free_semaphores(list(self.sems.allocated().values()))
        nc.all_engine_barrier()

    _drain_and_barrier._split_patched = True
    tile.TileContext._drain_and_barrier = _drain_and_barrier


def _split_excess_waits(nc, max_waits=1):
    """Walrus's codegen rejects instructions carrying more than one sync wait.
    Move excess waits onto pure-wait EventSemaphore instructions inserted
    immediately before the over-subscribed instruction on the same engine."""
    import bass_rust
    from concourse import mybir

    n_split = 0
    for f in nc.m.functions:
        for bb in f.blocks:
            lst = bb.instructions
            i = 0
            while i < len(lst):
                ins = lst[i]
                si = getattr(ins, "sync_info", None)
                if si is not None and len(si.on_wait) > max_waits:
                    waits = list(si.on_wait)
                    ins.sync_info = bass_rust.SyncInfo(
                        on_wait=waits[:max_waits], on_update=list(si.on_update)
                    )
                    for j, w in enumerate(waits[max_waits:]):
                        ev = mybir.InstEventSemaphore(
                            name=f"{ins.name}-xw{j}", ins=[], outs=[]
                        )
                        ev.engine = ins.engine
                        ev.sync_info = bass_rust.SyncInfo(on_wait=[w], on_update=[])
                        lst.insert(i, ev)
                        i += 1
                    n_split += 1
                i += 1
    return n_split


_NC_CACHE = None


def _build_module():
    global _NC_CACHE
    if _NC_CACHE is not None:
        return _NC_CACHE

    _install_ntff_hook_shim()
    _patch_tile_drain_split()

    import concourse.bass as bass
    import concourse.tile as tile
    from concourse import mybir

    f32 = mybir.dt.float32
    bf16 = mybir.dt.bfloat16
    Alu = mybir.AluOpType
    Act = mybir.ActivationFunctionType

    nc = bass.Bass("TRN2", target_bir_lowering=False, debug=False)
    bins_d = nc.dram_tensor("bins", [N_ROWS, N_BINS], f32, kind="ExternalInput").ap()
    px_d = nc.dram_tensor("px", [N_ROWS, 128, FREE], f32, kind="ExternalInput").ap()
    runmin_d = nc.dram_tensor(
        "runmin", [N_ROWS, 128, N_BINS], f32, kind="ExternalOutput"
    ).ap()
    nnsum_d = nc.dram_tensor("nnsum", [N_ROWS, 128, 1], f32, kind="ExternalOutput").ap()
    cnt_d = nc.dram_tensor("cnt", [N_ROWS, 128, 1], f32, kind="ExternalOutput").ap()

    # ScalarE produces |bins - y| tiles (Abs activation with per-partition
    # bias); VectorE does both reductions, batched K slots per instruction so
    # the 58-cycle DVE instruction overhead amortizes.
    KB = 25  # slots per batched DVE op; FREE must be divisible by KB

    with tile.TileContext(nc) as tc:
        with (
            tc.tile_pool(name="row", bufs=3) as row_pool,
            tc.tile_pool(name="acc", bufs=3) as acc_pool,
            tc.tile_pool(name="d2p", bufs=6) as d2_pool,
            tc.tile_pool(name="small", bufs=4) as small_pool,
        ):
            for r in range(N_ROWS):
                bins_bc = row_pool.tile([128, N_BINS], f32, tag="bins_bc")
                bins_row = bins_d[r]
                bins_bcast_ap = bass.AP(
                    tensor=bins_row.tensor,
                    offset=bins_row.offset,
                    ap=[[0, 128]] + list(bins_row.ap),
                )
                nc.sync.dma_start(out=bins_bc[:], in_=bins_bcast_ap)

                y = row_pool.tile([128, FREE], f32, tag="y")
                nc.sync.dma_start(out=y[:], in_=px_d[r])

                # mask = (y >= MIN_DEPTH) in {0.0, 1.0}
                mask = row_pool.tile([128, FREE], f32, tag="mask")
                nc.vector.tensor_scalar(
                    out=mask[:], in0=y[:], scalar1=MIN_DEPTH, scalar2=None,
                    op0=Alu.is_ge,
                )
                # y_off = y + OFF*(1-mask); nyo = -y_off
                t1 = small_pool.tile([128, FREE], f32, tag="t1")
                nc.vector.tensor_scalar(
                    out=t1[:], in0=mask[:], scalar1=OFF, scalar2=OFF,
                    op0=Alu.mult, op1=Alu.subtract,
                )
                nyo = row_pool.tile([128, FREE], f32, tag="nyo")
                nc.vector.tensor_tensor(out=nyo[:], in0=t1[:], in1=y[:], op=Alu.subtract)


                rm = acc_pool.tile([128, N_BINS], f32, tag="rm")
                nc.vector.memset(rm[:], BIG)
                nn_all = acc_pool.tile([128, FREE], f32, tag="nn_all")

                # Small leading batches on the first row shorten the pipeline
                # fill before VectorE gets its first work.
                schedule = [5, 10, 15, 20, 25] if r == 0 else [25, 25, 25]
                f0 = 0
                for bsz in schedule:
                    adw = d2_pool.tile([128, KB, N_BINS], f32, tag="adw")
                    for k in range(bsz):
                        f = f0 + k
                        nc.scalar.activation(
                            out=adw[:, k, :], in_=bins_bc[:], func=Act.Abs,
                            bias=nyo[:, f : f + 1], scale=1.0,
                        )
                    # per-pixel nearest-bin |d|: reduce innermost (bins) for
                    # the batch's slots in one DVE op
                    nc.vector.tensor_reduce(
                        out=nn_all[:, f0 : f0 + bsz], in_=adw[:, 0:bsz, :],
                        axis=mybir.AxisListType.X, op=Alu.min,
                    )
                    # per-bin running min: in-place pairwise halving tree over
                    # the batch's slots, then one fold into rm
                    n = bsz
                    while n > 1:
                        h = n // 2
                        nc.vector.tensor_tensor(
                            out=adw[:, 0:h, :], in0=adw[:, 0:h, :],
                            in1=adw[:, h : 2 * h, :], op=Alu.min,
                        )
                        if n % 2:
                            nc.vector.tensor_tensor(
                                out=adw[:, 0, :], in0=adw[:, 0, :],
                                in1=adw[:, n - 1, :], op=Alu.min,
                            )
                        n = h
                    nc.vector.tensor_tensor(
                        out=rm[:], in0=rm[:], in1=adw[:, 0, :], op=Alu.min
                    )
                    f0 += bsz

                # cham_y partials: nn^2 * mask summed over the free dim + count
                nn2 = small_pool.tile([128, FREE], f32, tag="nn2")
                nc.scalar.activation(out=nn2[:], in_=nn_all[:], func=Act.Square)
                nnm = small_pool.tile([128, FREE], f32, tag="nnm")
                nc.vector.tensor_tensor(out=nnm[:], in0=nn2[:], in1=mask[:], op=Alu.mult)
                nnsum = small_pool.tile([128, 1], f32, tag="nnsum")
                nc.vector.tensor_reduce(
                    out=nnsum[:], in_=nnm[:], axis=mybir.AxisListType.X, op=Alu.add
                )
                cnt = small_pool.tile([128, 1], f32, tag="cnt")
                nc.vector.tensor_reduce(
                    out=cnt[:], in_=mask[:], axis=mybir.AxisListType.X, op=Alu.add
                )

                nc.sync.dma_start(out=runmin_d[r], in_=rm[:])
                nc.sync.dma_start(out=nnsum_d[r], in_=nnsum[:])
                nc.sync.dma_start(out=cnt_d[r], in_=cnt[:])

    _split_excess_waits(nc)
    _NC_CACHE = nc
    return nc


LAST_RESULTS = None


def kernel(bin_centers: np.ndarray, target_depth_maps: np.ndarray) -> np.ndarray:
    global LAST_RESULTS
    nc = _build_module()
    from concourse import bass_utils

    trace = bool(os.environ.get("KERNEL_TRACE"))
    if trace:
        bass_utils.upload_artifacts = lambda tmpdir: "local://" + str(tmpdir)

    bins = np.ascontiguousarray(bin_centers, dtype=np.float32)
    tp = np.ascontiguousarray(
        np.asarray(target_depth_maps, dtype=np.float32).reshape(N_ROWS, HW)
    )

    in_maps = []
    for c in range(N_CORES):
        sl = tp[:, c * PX_PER_CORE : (c + 1) * PX_PER_CORE].reshape(N_ROWS, 128, FREE)
        in_maps.append({"bins": bins, "px": np.ascontiguousarray(sl)})

    res = bass_utils.run_bass_kernel_spmd(
        nc, in_maps, core_ids=list(range(N_CORES)), trace=trace
    )
    LAST_RESULTS = res

    runmin = np.stack([r["runmin"] for r in res.results])  # [8, 4, 128, 256]
    nnsum = np.stack([r["nnsum"] for r in res.results])    # [8, 4, 128, 1]
    cnt = np.stack([r["cnt"] for r in res.results])        # [8, 4, 128, 1]

    # runmin holds |d|; square in fp32 (monotone => same as min over fp32 d^2),
    # then clamp to BIG to reproduce the reference's invalid-pixel sentinel.
    per_bin_absd = runmin.min(axis=(0, 2)).astype(np.float32)       # [4, 256]
    per_bin = np.minimum(per_bin_absd * per_bin_absd, np.float32(BIG))
    cham_x = per_bin.mean(axis=1, dtype=np.float64)                 # [4]
    lengths = cnt.sum(axis=(0, 2, 3), dtype=np.float64)             # [4]
    sums = nnsum.sum(axis=(0, 2, 3), dtype=np.float64)              # [4]
    cham_y = sums / np.maximum(lengths, 1.0)
    out = np.mean(cham_x + cham_y)
    return np.asarray(out, dtype=np.float32)
